# revision 1
# baseline (speedup 1.0000x reference)
"""Trainium2 Bass kernel for VITS-style relative-position MultiHeadAttention.

Problem: B=4, T=1024, C=512, H=8 heads, d=64, window=4 relative attention
(rel embeddings shared across heads). Sharded over 8 NeuronCores as
(batch x head-group): core = 2*b + hg, each core handles batch b and 4 heads.

v3 changes vs baseline:
  - all matmuls run as float32r (1 PE cycle/row vs 4 for fp32)
  - band add into scores via PE identity-matmul accumulate (bf16 win)
    instead of DVE tensor_tensor (saves ~12us DVE)
  - rel-K logits computed for head PAIRS with a block-diagonal [128,18]
    stationary (halves phase-B matmul columns)
  - phase-B/C transposes batched into single PSUM tiles -> one DVE
    evacuation each (cuts PSUM-access init overhead)
  - softmax normalization folded into PV evacuation (reciprocal +
    partition-broadcast multiply); output projection contracts the full
    128-row head pair per matmul and accumulates both chunks in PSUM
    (halves phase-D matmul columns, removes 32 DVE ops)
"""

import numpy as np

import concourse.bass as bass
import concourse.bacc as bacc
import concourse.mybir as mybir
import concourse.tile as tile
from concourse.bass_utils import run_bass_kernel_spmd
from concourse.masks import make_identity

f32 = mybir.dt.float32
f32r = mybir.dt.float32r
bf16 = mybir.dt.bfloat16
i16 = mybir.dt.int16

T = 1024          # sequence length (t_t == t_s)
CIN = 512         # input channels
CH = 256          # channels per core (head group)
NHEADS = 4        # heads per core
D = 64            # head dim
NB = 9            # band width (2*window+1)
NT = T // 128     # 8 tiles of 128
GPITCH = 137      # G buffer row pitch (136 + 1)
GSZ = T * GPITCH + 256

Exp = mybir.ActivationFunctionType.Exp
Identity = mybir.ActivationFunctionType.Identity
Copy = mybir.ActivationFunctionType.Copy
AluAdd = mybir.AluOpType.add
AluMult = mybir.AluOpType.mult


def build_program():
    nc = bacc.Bacc()

    # fp32r matmul: fp32 data, 1 PE cycle/row when moving dim >= 256
    def mmr(out, lhsT, rhs, **kw):
        nc.tensor.matmul(out, lhsT.bitcast(f32r), rhs.bitcast(f32r), **kw)

    def trp(out, in_, identity):
        nc.tensor.matmul(out, in_, identity, is_transpose=True)

    # ---- external I/O (per-core shapes) ----
    xT = nc.declare_dram_parameter("xT", [CIN, T], f32r, isOutput=False)
    cT = nc.declare_dram_parameter("cT", [CIN, T], f32r, isOutput=False)
    wq = nc.declare_dram_parameter("wq", [CIN, CH], f32r, isOutput=False)
    wk = nc.declare_dram_parameter("wk", [CIN, CH], f32r, isOutput=False)
    wv = nc.declare_dram_parameter("wv", [CIN, CH], f32r, isOutput=False)
    wo = nc.declare_dram_parameter("wo", [CH, CIN], f32r, isOutput=False)
    bq2 = nc.declare_dram_parameter("bq2", [128, 2], f32, isOutput=False)
    bk2 = nc.declare_dram_parameter("bk2", [128, 2], f32, isOutput=False)
    bv1 = nc.declare_dram_parameter("bv1", [1, CH], f32r, isOutput=False)
    ek2p = nc.declare_dram_parameter("ek2p", [128, 32 + NB], f32r, isOutput=False)
    ev65 = nc.declare_dram_parameter("ev65", [NB, D + 1], f32r, isOutput=False)
    ones128 = nc.declare_dram_parameter("ones128", [1, 128], f32r, isOutput=False)
    vcol4 = nc.declare_dram_parameter("vcol4", [128, NHEADS], f32r, isOutput=False)
    zz64 = nc.declare_dram_parameter("zz64", [128, 64], f32r, isOutput=False)
    e2p = nc.declare_dram_parameter("e2p", [2, 128], f32r, isOutput=False)
    sidx = nc.declare_dram_parameter("sidx", [128, 10], i16, isOutput=False)
    out_p = nc.declare_dram_parameter("out_p", [T, CIN], f32, isOutput=True)

    with tile.TileContext(nc) as tc:
        with (
            tc.tile_pool(name="const", bufs=1) as cpool,
            tc.tile_pool(name="win", bufs=1) as wpool,
            tc.tile_pool(name="xin", bufs=1) as xpool,
            tc.tile_pool(name="qk", bufs=1) as qkpool,
            tc.tile_pool(name="vaug", bufs=1) as vpool,
            tc.tile_pool(name="band", bufs=1) as bpool,
            tc.tile_pool(name="et", bufs=10) as etpool,
            tc.tile_pool(name="outp", bufs=1) as opool,
            tc.tile_pool(name="dram", bufs=1, space="DRAM") as dpool,
        ):
            # ---------- constants ----------
            ident = cpool.tile([128, 128], f32)
            make_identity(nc, ident[:])
            identb = cpool.tile([128, 128], bf16)
            make_identity(nc, identb[:])
            ones1 = cpool.tile([1, 128], f32r)
            nc.sync.dma_start(ones1[:], ones128[:])
            e2_sb = cpool.tile([2, 128], f32r)
            nc.sync.dma_start(e2_sb[:], e2p[:])
            sidx_sb = cpool.tile([128, 10], i16)
            nc.sync.dma_start(sidx_sb[:], sidx[:])
            # block-diag rel-K stationary: head-pair in one matmul
            ek2 = cpool.tile([128, 32 + NB], f32r)
            nc.sync.dma_start(ek2[:], ek2p[:])
            ev_sb = cpool.tile([NB, D + 1], f32r)
            nc.sync.dma_start(ev_sb[:], ev65[:])
            bq_sb = cpool.tile([128, 2], f32)
            nc.sync.dma_start(bq_sb[:], bq2[:])
            bk_sb = cpool.tile([128, 2], f32)
            nc.sync.dma_start(bk_sb[:], bk2[:])
            bv_sb = cpool.tile([1, CH], f32r)
            nc.sync.dma_start(bv_sb[:], bv1[:])

            # ---------- load weights + inputs ----------
            wq_sb = []
            wk_sb = []
            wv_sb = []
            xT_sb = []
            cT_sb = []
            for kt in range(4):
                t_ = wpool.tile([128, CH], f32r, tag=f"wq{kt}")
                nc.sync.dma_start(t_[:], wq[kt * 128:(kt + 1) * 128, :])
                wq_sb.append(t_)
                t_ = wpool.tile([128, CH], f32r, tag=f"wk{kt}")
                nc.sync.dma_start(t_[:], wk[kt * 128:(kt + 1) * 128, :])
                wk_sb.append(t_)
                t_ = wpool.tile([128, CH], f32r, tag=f"wv{kt}")
                nc.sync.dma_start(t_[:], wv[kt * 128:(kt + 1) * 128, :])
                wv_sb.append(t_)
                t_ = xpool.tile([128, T], f32r, tag=f"xT{kt}")
                nc.sync.dma_start(t_[:], xT[kt * 128:(kt + 1) * 128, :])
                xT_sb.append(t_)
                t_ = xpool.tile([128, T], f32r, tag=f"cT{kt}")
                nc.sync.dma_start(t_[:], cT[kt * 128:(kt + 1) * 128, :])
                cT_sb.append(t_)
            wo_sb = []
            for ct in range(2):
                t_ = wpool.tile([128, CIN], f32r, tag=f"wo{ct}")
                nc.sync.dma_start(t_[:], wo[ct * 128:(ct + 1) * 128, :])
                wo_sb.append(t_)

            # ---------- phase A: QKV projections ----------
            qsT_sb = [qkpool.tile([128, T], f32r, tag=f"qsT{ct}", name=f"qsT{ct}") for ct in range(2)]
            kT_sb = [qkpool.tile([128, T], f32r, tag=f"kT{ct}", name=f"kT{ct}") for ct in range(2)]
            with tc.tile_pool(name="psA", bufs=4, space="PSUM") as psA:
                for ct in range(2):
                    for nh in range(2):
                        ps = psA.tile([128, 512], f32, tag="qk")
                        for kt in range(4):
                            mmr(
                                ps[:],
                                wq_sb[kt][:, ct * 128:(ct + 1) * 128],
                                xT_sb[kt][:, nh * 512:(nh + 1) * 512],
                                start=(kt == 0), stop=(kt == 3),
                            )
                        # q_scaled = (x@Wq)*0.125 + bq*0.125  (bq2 pre-scaled)
                        nc.scalar.activation(
                            qsT_sb[ct][:, nh * 512:(nh + 1) * 512], ps[:],
                            Identity, bias=bq_sb[:, ct:ct + 1], scale=0.125,
                        )
                        ps = psA.tile([128, 512], f32, tag="qk")
                        for kt in range(4):
                            mmr(
                                ps[:],
                                wk_sb[kt][:, ct * 128:(ct + 1) * 128],
                                cT_sb[kt][:, nh * 512:(nh + 1) * 512],
                                start=(kt == 0), stop=(kt == 3),
                            )
                        # k = psum + bk on DVE (balances Act engine load)
                        nc.vector.tensor_scalar(
                            kT_sb[ct][:, nh * 512:(nh + 1) * 512], ps[:],
                            bk_sb[:, ct:ct + 1], None, op0=AluAdd,
                        )
                # v natural [s, ch] + ones column per head -> [128, 4*65]
                vaug_sb = []
                for st in range(NT):
                    va = vpool.tile([128, NHEADS * (D + 1)], f32r, tag=f"va{st}")
                    nc.sync.dma_start(
                        va[:].rearrange("p (h c) -> p h c", h=NHEADS)[:, :, D:D + 1].squeeze(axis=2),
                        vcol4[:],
                    )
                    ps = psA.tile([128, CH], f32, tag="v")
                    for kt in range(4):
                        mmr(
                            ps[:],
                            cT_sb[kt][:, st * 128:(st + 1) * 128],
                            wv_sb[kt][:],
                            start=(kt == 0), stop=False,
                        )
                    mmr(ps[:], ones1[:], bv_sb[:], start=False, stop=True)
                    nc.vector.tensor_copy(
                        va[:].rearrange("p (h c) -> p h c", h=NHEADS)[:, :, 0:D],
                        ps[:].rearrange("p (h c) -> p h c", h=NHEADS),
                    )
                    vaug_sb.append(va)

            # ---------- DRAM bounce tiles for the skew transports ----------
            rld_dram = [dpool.tile([NB, T + 8], f32, tag=f"rld{h}", name=f"rld{h}") for h in range(NHEADS)]
            atd_dram = [dpool.tile([NB, T + 8], f32, tag=f"atd{h}", name=f"atd{h}") for h in range(NHEADS)]
            zb_sb = cpool.tile([NB, 4], f32)
            nc.sync.dma_start(zb_sb[:], zz64[0:NB, 0:4].bitcast(f32))

            # ---------- phase B: rel-K band prep ----------
            # RL9T[j, t] = emb_k[j] . q_scaled[t] for head PAIRS -> skewed S
            s4t_cat = bpool.tile([64, T], f32, tag="s4t")
            nc.gpsimd.memset(s4t_cat[:], 0.0)
            sbf_all = bpool.tile([128, NT * NHEADS * 10], bf16, tag="sbfall")
            nc.gpsimd.memset(sbf_all[:], 0.0)
            with tc.tile_pool(name="psB", bufs=2, space="PSUM") as psB:
                for ct in range(2):
                    rl = psB.tile([32 + NB, T], f32, tag="rl18")
                    for nh in range(2):
                        mmr(
                            rl[:, nh * 512:(nh + 1) * 512],
                            ek2[:],
                            qsT_sb[ct][:, nh * 512:(nh + 1) * 512],
                            start=True, stop=True,
                        )
                    for hh in range(2):
                        h = 2 * ct + hh
                        rlp = bpool.tile([NB, T], f32, tag=f"rlp{h}")
                        nc.vector.tensor_copy(
                            rlp[:], rl[hh * 32:hh * 32 + NB, :]
                        )
                        # DRAM bounce with diagonal AP replaces 9 row DMAs:
                        # rld[r, 4:T+4] = RL9T[r, :], zero borders, then
                        # s4t'[h*16+r, c] = rld[r, 8-r+c]  (stride T+7; the
                        # row flip vs the old layout is compensated by the
                        # host-side sidx change p+j -> p+8-j)
                        rld = rld_dram[h]
                        nc.sync.dma_start(
                            bass.AP(rld[:].tensor, rld[:].offset,
                                    [[T + 8, NB], [1, 4]]),
                            bass.AP(zb_sb[:].tensor, zb_sb[:].offset,
                                    [[4, NB], [1, 4]]),
                        )
                        nc.sync.dma_start(
                            bass.AP(rld[:].tensor, rld[:].offset + T + 4,
                                    [[T + 8, NB], [1, 4]]),
                            bass.AP(zb_sb[:].tensor, zb_sb[:].offset,
                                    [[4, NB], [1, 4]]),
                        )
                        nc.sync.dma_start(
                            bass.AP(rld[:].tensor, rld[:].offset + 4,
                                    [[T + 8, NB], [1, T]]),
                            rlp[:],
                        )
                        nc.sync.dma_start(
                            s4t_cat[h * 16:h * 16 + NB, :],
                            bass.AP(rld[:].tensor, rld[:].offset + 8,
                                    [[T + 7, NB], [1, T]]),
                        )
                # transpose to S [128(s), .]; batched psum, single bf16 copy
                pst = psB.tile([128, 512], f32, tag="s4tp")
                for st in range(NT):
                    trp(
                        pst[:, st * 64:(st + 1) * 64],
                        s4t_cat[:, st * 128:(st + 1) * 128],
                        ident[0:64, 0:64],
                    )
                # col layout: (st*4+h)*16 + c  ->  (st*4+h)*10 + c
                nc.vector.tensor_copy(
                    sbf_all[:].rearrange("p (g c) -> p g c", g=32)[:, :, 0:NB],
                    pst[:].rearrange("p (g c) -> p g c", g=32)[:, :, 0:NB],
                )

            # G bounce buffers (skewed band storage), one per head
            g_dram = [dpool.tile([1, GSZ], f32r, tag=f"g{h}", name=f"g{h}") for h in range(NHEADS)]
            zeros_sb = cpool.tile([1, 40], f32r)
            nc.sync.dma_start(zeros_sb[:], zz64[0:1, 0:40])
            for h in range(NHEADS):
                gt = g_dram[h]
                # zero the band cells of rows 0..3 and 1020..1023 (t out of range)
                nc.sync.dma_start(
                    bass.AP(gt[:].tensor, gt[:].offset, [[GPITCH, 4], [1, NB]]),
                    bass.AP(zeros_sb[:].tensor, zeros_sb[:].offset, [[NB, 4], [1, NB]]),
                )
                nc.sync.dma_start(
                    bass.AP(gt[:].tensor, gt[:].offset + 1020 * GPITCH,
                            [[GPITCH, 4], [1, NB]]),
                    bass.AP(zeros_sb[:].tensor, zeros_sb[:].offset, [[NB, 4], [1, NB]]),
                )

            # ---------- phase C: per-head attention ----------
            outT_sb = [opool.tile([128, T], f32r, tag=f"oT{ct}", name=f"oT{ct}") for ct in range(2)]
            abs4 = [bpool.tile([128, 64], f32r, tag=f"abs4_{st}", name=f"abs4_{st}") for st in range(NT)]
            for st in range(NT):
                nc.sync.dma_start(abs4[st][:], zz64[:])
            with (
                tc.tile_pool(name="psS", bufs=2, space="PSUM") as psS,
                tc.tile_pool(name="psPV", bufs=1, space="PSUM") as psPV,
                tc.tile_pool(name="psT", bufs=1, space="PSUM") as psT,
            ):
                pvraw = []
                recs = {}
                dsums = [bpool.tile([2, T], f32, tag=f"ds{i}", name=f"ds{i}")
                         for i in range(2)]
                for h in range(NHEADS):
                    ct, r0 = h // 2, (h % 2) * 64
                    pv = psPV.tile([D + 1, T], f32, tag="pv")
                    et_tiles = []
                    for st in range(NT):
                        s0 = st * 128
                        sc = psS.tile([128, T], f32, tag="sc")
                        for nh in range(2):
                            mmr(
                                sc[:, nh * 512:(nh + 1) * 512],
                                kT_sb[ct][r0:r0 + 64, s0:s0 + 128],
                                qsT_sb[ct][r0:r0 + 64, nh * 512:(nh + 1) * 512],
                                start=True, stop=True,
                            )
                        # band add via PE identity-matmul accumulate
                        win = bpool.tile([128, 136], bf16, tag="win")
                        nc.gpsimd.local_scatter(
                            win[:], sbf_all[:, (st * 4 + h) * 10:(st * 4 + h) * 10 + 10],
                            sidx_sb[:], channels=128, num_elems=136, num_idxs=10,
                        )
                        lo = 4 if st == 0 else 0
                        hi = 132 if st == NT - 1 else 136
                        c = lo
                        while c < hi:
                            col = s0 - 4 + c
                            nxt = min(hi, c + (512 - (col % 512)))
                            nc.tensor.matmul(
                                sc[:, col:col + (nxt - c)],
                                identb[:], win[:, c:nxt],
                                start=False, stop=True, skip_group_check=True,
                            )
                            c = nxt
                        et = etpool.tile([128, T], f32r, tag="et")
                        nc.scalar.activation(et[:], sc[:], Exp)
                        et_tiles.append(et)
                        # band window -> G (contiguous 544B runs, skewed layout)
                        gt = g_dram[h]
                        nc.sync.dma_start(
                            bass.AP(gt[:].tensor,
                                    gt[:].offset + s0 * GPITCH + lo,
                                    [[136, 128], [1, hi - lo]]),
                            et[:, s0 - 4 + lo:s0 - 4 + hi],
                        )
                        # compact band readback [128, 9] (36B runs)
                        nc.sync.dma_start(
                            abs4[st][:, h * 16:h * 16 + NB],
                            bass.AP(gt[:].tensor, gt[:].offset + s0 * GPITCH,
                                    [[GPITCH, 128], [1, NB]]),
                        )
                    # PV: out^T[d, t] (+ colsum in row 64) accumulated over s
                    for st in range(NT):
                        for nh in range(2):
                            mmr(
                                pv[:, nh * 512:(nh + 1) * 512],
                                vaug_sb[st][:, h * 65:h * 65 + 65],
                                et_tiles[st][:, nh * 512:(nh + 1) * 512],
                                start=(st == 0), stop=False,
                            )
                    # rel-V: Aband'[j, t] via DRAM diagonal skew
                    pat = psT.tile([64, T], f32, tag="pat")
                    for st in range(NT):
                        trp(pat[:, st * 128:(st + 1) * 128], abs4[st][:, :].bitcast(f32), ident[:])
                    at_cat = bpool.tile([64, T], f32, tag=f"atc{h % 2}")
                    nc.vector.tensor_copy(at_cat[:], pat[:])
                    atd = atd_dram[h]
                    nc.sync.dma_start(
                        bass.AP(atd[:].tensor, atd[:].offset,
                                [[T + 8, NB], [1, 4]]),
                        bass.AP(zb_sb[:].tensor, zb_sb[:].offset,
                                [[4, NB], [1, 4]]),
                    )
                    nc.sync.dma_start(
                        bass.AP(atd[:].tensor, atd[:].offset + T + 4,
                                [[T + 8, NB], [1, 4]]),
                        bass.AP(zb_sb[:].tensor, zb_sb[:].offset,
                                [[4, NB], [1, 4]]),
                    )
                    nc.sync.dma_start(
                        bass.AP(atd[:].tensor, atd[:].offset + 4,
                                [[T + 8, NB], [1, T]]),
                        at_cat[h * 16:h * 16 + NB, :],
                    )
                    # abt'[m, c] = at_cat[h*16+m, c+4-m] = atd[m, c+8-m]
                    abt = bpool.tile([NB, T], f32r, tag=f"abt{h % 2}")
                    nc.sync.dma_start(
                        abt[:],
                        bass.AP(atd[:].tensor, atd[:].offset + 8,
                                [[T + 7, NB], [1, T]]).bitcast(f32r),
                    )
                    for nh in range(2):
                        mmr(
                            pv[:, nh * 512:(nh + 1) * 512],
                            ev_sb[:],
                            abt[:, nh * 512:(nh + 1) * 512],
                            start=False, stop=True,
                        )
                    # evacuate raw (frees pv fast); normalization in tail
                    pvr = opool.tile([D + 1, T], f32, tag=f"pvr{h}", name=f"pvr{h}")
                    nc.vector.tensor_copy(pvr[:], pv[:])
                    pvraw.append(pvr)
                    nc.sync.dma_start(dsums[h // 2][h % 2:h % 2 + 1, :],
                                      pvr[D:D + 1, :])
                    if h % 2 == 1:
                        # per-pair batched reciprocal: pair 0's runs while
                        # heads 2-3 still compute
                        pr = h // 2
                        rf = bpool.tile([2, T], f32, tag="rcf")
                        nc.vector.reciprocal(rf[:], dsums[pr][:])
                        rc = bpool.tile([2, T], f32r, tag=f"rc{pr}")
                        nc.vector.tensor_copy(rc[:], rf[:])
                        recs[pr] = rc

                # tail: selector matmul broadcasts each pair's recip rows
                # across 128 partitions per ct
                for ct in range(2):
                    rb = psT.tile([128, T], f32, tag="pat")
                    for nh in range(2):
                        mmr(rb[:, nh * 512:(nh + 1) * 512],
                            e2_sb[:],
                            recs[ct][:, nh * 512:(nh + 1) * 512],
                            start=True, stop=True)
                    for hh in range(2):
                        h, r0 = 2 * ct + hh, hh * 64
                        nc.vector.tensor_tensor(
                            outT_sb[ct][r0:r0 + 64, :], pvraw[h][0:D, :],
                            rb[r0:r0 + 64, :], op=AluMult,
                        )

            # ---------- phase D: output projection (heads pre-normalized) ----------
            with tc.tile_pool(name="psP", bufs=2, space="PSUM") as psP:
                for st in range(NT):
                    pp = psP.tile([128, CIN], f32, tag="pj")
                    for ct in range(2):
                        mmr(
                            pp[:],
                            outT_sb[ct][:, st * 128:(st + 1) * 128],
                            wo_sb[ct][:],
                            start=(ct == 0), stop=(ct == 1),
                        )
                    acc = opool.tile([128, CIN], f32, tag=f"acc{st % 2}")
                    if st % 2 == 0:
                        nc.scalar.activation(acc[:], pp[:], Copy)
                    else:
                        nc.vector.tensor_copy(acc[:], pp[:])
                    nc.sync.dma_start(out_p[st * 128:(st + 1) * 128, :], acc[:])

    nc.compile()
    return nc


def make_core_inputs(x, c, Wq, bq, Wk, bk, Wv, bv, Wo, bo, emb_rel_k, emb_rel_v,
                     core):
    b, hg = core // 2, core % 2
    sl = slice(hg * CH, (hg + 1) * CH)
    ek2p = np.zeros((128, 32 + NB), np.float32)
    ek2p[0:D, 0:NB] = emb_rel_k[0].T
    ek2p[D:2 * D, 32:32 + NB] = emb_rel_k[0].T
    e2p = np.zeros((2, 128), np.float32)
    e2p[0, 0:64] = 1.0
    e2p[1, 64:128] = 1.0
    ev65 = np.zeros((NB, D + 1), np.float32)
    ev65[:, 0:D] = emb_rel_v[0][::-1]
    si = np.zeros((128, 10), np.int16)
    for p in range(128):
        for j in range(NB):
            si[p, j] = p + 8 - j
        si[p, 9] = -1
    return {
        "xT": np.ascontiguousarray(x[b].T).astype(np.float32),
        "cT": np.ascontiguousarray(c[b].T).astype(np.float32),
        "wq": np.ascontiguousarray(Wq[:, sl]).astype(np.float32),
        "wk": np.ascontiguousarray(Wk[:, sl]).astype(np.float32),
        "wv": np.ascontiguousarray(Wv[:, sl]).astype(np.float32),
        "wo": np.ascontiguousarray(Wo[sl, :]).astype(np.float32),
        "bq2": np.ascontiguousarray((bq[sl] * 0.125).reshape(2, 128).T).astype(np.float32),
        "bk2": np.ascontiguousarray(bk[sl].reshape(2, 128).T).astype(np.float32),
        "bv1": bv[sl].reshape(1, CH).astype(np.float32),
        "ek2p": ek2p,
        "ev65": ev65,
        "ones128": np.ones((1, 128), np.float32),
        "vcol4": np.ones((128, NHEADS), np.float32),
        "zz64": np.zeros((128, 64), np.float32),
        "e2p": e2p,
        "sidx": si,
    }


def kernel(**inputs):
    inputs = {k: np.asarray(v) for k, v in inputs.items()}
    nc = build_program()
    core_ids = list(range(8))
    in_maps = [make_core_inputs(core=i, **inputs) for i in core_ids]
    res = run_bass_kernel_spmd(nc, in_maps, core_ids).results
    B = inputs["x"].shape[0]
    out = np.zeros((B, T, CIN), np.float32)
    for b in range(B):
        out[b] = res[2 * b]["out_p"] + res[2 * b + 1]["out_p"] + inputs["bo"]
    return out

